# revision 1
# baseline (speedup 1.0000x reference)
"""Trainium2 Bass kernel for nn_NeRF_MLP_Compose (MoE-routed NeRF MLP).

Strategy:
  - Host-side MoE dispatch (the sharding step): rows are permuted so each of
    the 8 cores receives a fixed-capacity, expert-contiguous block of rows
    (4 experts x 2304 rows, padded).  Each core then runs a dense per-expert
    MLP over its rows; outputs are inverse-permuted on the host.
  - All math (x normalize, positional encoding, 5 matmul layers, residuals,
    final division) runs on device.
  - Device layout: activations transposed (features on partitions, rows on
    the free dimension).  Positional encoding: theta built by a small
    "selection matmul" (freqs folded into the selection matrix), range
    reduction via DVE mod ops, ACT Sin.
"""
import sys
for _p in ("/opt/trn_rl_repo", "/root/.axon_site/_ro/trn_rl_repo"):
    if _p not in sys.path:
        sys.path.insert(0, _p)

import numpy as np

N = 65536
E = 4            # experts
NCORE = 8
CAP = 2304       # rows per expert per core (18 * 128); global 18432 >> E[16384]
ROWS_CORE = E * CAP          # 9216
NUM_FREQS = 10
HID = 256
DOUT = 64
NL = 4           # layers -> 3 residual blocks
TWO_PI = float(2 * np.pi)
TWO_PI_F32 = float(np.float32(2 * np.pi))
MAGIC_C = float(np.float32(1.5 * 2 ** 23))
CLAMP_HI = float(np.float32(0.5) - np.float32(2 ** -25))

_compiled = {}
RUN_KWARGS = {}    # test.py may set e.g. {"trace": True}
LAST_RESULT = []   # test.py reads the BassKernelResults appended here


def _freqs_f32():
    return (2.0 ** np.arange(NUM_FREQS, dtype=np.float32)) * np.float32(np.pi)


def _build_program():
    import concourse.bass as bass
    from concourse import bacc
    import concourse.mybir as mybir
    import concourse.tile as tile
    from concourse.masks import make_identity

    F32 = mybir.dt.float32
    F32R = mybir.dt.float32r
    P = 128

    nc = bacc.Bacc("TRN2", target_bir_lowering=False, debug=False)

    # ---- DRAM I/O ----
    x_d = nc.dram_tensor("x_rows", [ROWS_CORE, 4], F32, kind="ExternalInput").ap()
    d_d = nc.dram_tensor("indim_rows", [ROWS_CORE], F32, kind="ExternalInput").ap()
    bsel_d = nc.dram_tensor("bsel", [5, 80], F32, kind="ExternalInput").ap()
    w0a_d = nc.dram_tensor("w0a", [4, E, HID], F32, kind="ExternalInput").ap()
    w0b_d = nc.dram_tensor("w0b", [80, E, HID], F32, kind="ExternalInput").ap()
    wh_d = nc.dram_tensor("wh", [P, E, NL - 1, 2, HID], F32, kind="ExternalInput").ap()
    wo_d = nc.dram_tensor("wo", [P, E, 2, DOUT], F32, kind="ExternalInput").ap()
    b0_d = nc.dram_tensor("b0r", [P, E, 2], F32, kind="ExternalInput").ap()
    bh_d = nc.dram_tensor("bhr", [P, E, NL - 1, 2], F32, kind="ExternalInput").ap()
    bo_d = nc.dram_tensor("bor", [DOUT, E], F32, kind="ExternalInput").ap()
    sc_d = nc.dram_tensor("scal12", [E * (NL - 1)], F32, kind="ExternalInput").ap()
    out_d = nc.dram_tensor("out_rows", [ROWS_CORE, DOUT], F32,
                           kind="ExternalOutput").ap()

    with tile.TileContext(nc) as tc:
        with tc.tile_pool(name="const", bufs=1) as cpool, \
             tc.tile_pool(name="work", bufs=3) as wpool, \
             tc.tile_pool(name="hbuf", bufs=3) as hpool, \
             tc.tile_pool(name="psA", bufs=1, space="PSUM") as psA, \
             tc.tile_pool(name="psB", bufs=2, space="PSUM") as psB:

            # ---- constants / weights into SBUF (once) ----
            ident = cpool.tile([P, P], F32)
            make_identity(nc, ident)
            bsel = cpool.tile([5, 80], F32)
            nc.sync.dma_start(out=bsel, in_=bsel_d)
            zero80 = cpool.tile([80, 1], F32)
            nc.vector.memset(zero80, 0.0)
            w0a = cpool.tile([4, E, HID], F32R)
            nc.gpsimd.dma_start(out=w0a, in_=w0a_d)
            w0b = cpool.tile([80, E, HID], F32R)
            nc.gpsimd.dma_start(out=w0b, in_=w0b_d)
            wh = cpool.tile([P, E, NL - 1, 2, HID], F32R)
            nc.gpsimd.dma_start(out=wh, in_=wh_d)
            wo = cpool.tile([P, E, 2, DOUT], F32R)
            nc.gpsimd.dma_start(out=wo, in_=wo_d)
            b0 = cpool.tile([P, E, 2], F32)
            nc.sync.dma_start(out=b0, in_=b0_d)
            bh = cpool.tile([P, E, NL - 1, 2], F32)
            nc.sync.dma_start(out=bh, in_=bh_d)
            bo = cpool.tile([DOUT, E], F32)
            nc.sync.dma_start(out=bo, in_=bo_d)
            scl = cpool.tile([P, E * (NL - 1)], F32)
            nc.sync.dma_start(
                out=scl,
                in_=bass.AP(tensor=sc_d.tensor, offset=0,
                            ap=[[0, P], [1, E * (NL - 1)]]))
            # s3-prescaled output weights: out = Wo^T h2 + (s3 Wo)^T t3,
            # which removes the third residual STT from the per-tile loop
            wos = cpool.tile([P, E, 2, DOUT], F32R)
            for ee in range(E):
                nc.vector.tensor_scalar_mul(
                    wos[:, ee, :, :], wo[:, ee, :, :],
                    scl[:, ee * (NL - 1) + 2:ee * (NL - 1) + 3])

            def do_tile(e, r0, R):
                c = R // P
                # loads
                x_t = wpool.tile([P, 4, 4], F32, tag="x_t")
                nc.sync.dma_start(
                    out=x_t[:, :c, :],
                    in_=bass.AP(tensor=x_d.tensor, offset=r0 * 4,
                                ap=[[4, P], [4 * P, c], [1, 4]]))
                d_t = wpool.tile([P, 4], F32, tag="d_t")
                nc.sync.dma_start(
                    out=d_t[:, :c],
                    in_=bass.AP(tensor=d_d.tensor, offset=r0,
                                ap=[[1, P], [P, c]]))

                # normalize: xn = x * (1/x3), reciprocal + one Newton step
                # (walrus has no divide ALU op), then restore x3
                rc0 = wpool.tile([P, 4], F32, tag="rc0")
                nc.vector.reciprocal(rc0[:, :c], x_t[:, :c, 3])
                xn = wpool.tile([P, 4, 5], F32, tag="xn")
                nc.vector.tensor_mul(xn[:, :c, 0:4], x_t[:, :c, :],
                                     rc0[:, :c, None].to_broadcast((P, c, 4)))
                nc.vector.tensor_copy(xn[:, :c, 3], x_t[:, :c, 3])
                nc.vector.memset(xn[:, :c, 4], 1.0)

                # transpose -> xnT [5, R]
                ps_x4 = psA.tile([5, 4, P], F32, tag="x4o")
                for ch in range(c):
                    nc.tensor.transpose(ps_x4[:, ch, :], xn[:, ch, :], ident)
                xnT = wpool.tile([5, 512], F32, tag="xnT")
                nc.scalar.copy(xnT[:, :R], ps_x4[:, :c, :].rearrange("p c q -> p (c q)"))
                # f32r copy of x' rows for the layer-0 K=4 matmul
                x4r = wpool.tile([4, 512], F32R, tag="x4r")
                nc.scalar.copy(x4r[:, :R], xnT[0:4, :R])

                # t5 = Bsel^T xnT5: per row, t + phi_turn where t = x'*2^(i-1)
                # is EXACT (power-of-two freqs in turns); phi_turn = 0.25 on
                # cos rows implements the pi/2 phase shift.
                ps_t5 = psA.tile([80, 512], F32, tag="t5")
                nc.tensor.matmul(ps_t5[:, :R], bsel, xnT[:, :R],
                                 start=True, stop=True)
                # k = round(t5) via the fp32 magic-add trick, on DVE;
                # m0 = t5 - k in [-.5-eps, .5+eps]; HW ACT clamps the rare
                # eps overshoot at the Sin input range boundary.
                kt = wpool.tile([80, 512], F32, tag="kt")
                nc.vector.tensor_scalar(kt[:, :R], ps_t5[:, :R], MAGIC_C,
                                        MAGIC_C, mybir.AluOpType.add,
                                        mybir.AluOpType.subtract)
                m0 = wpool.tile([80, 512], F32, tag="m0")
                nc.vector.scalar_tensor_tensor(m0[:, :R], kt[:, :R], -1.0,
                                               ps_t5[:, :R],
                                               mybir.AluOpType.mult,
                                               mybir.AluOpType.add)
                xe = wpool.tile([80, 512], F32R, tag="xe")
                nc.scalar.activation(xe[:, :R], m0[:, :R],
                                     mybir.ActivationFunctionType.Sin,
                                     bias=zero80, scale=TWO_PI_F32)

                # layer 0: z0 = W0a^T xnT + W0b^T xe ; h0 = relu(z0 + b0)
                ps_z = psB.tile([P, 2, 512], F32, tag="z")
                for mb in range(2):
                    nc.tensor.matmul(ps_z[:, mb, :R],
                                     w0a[:, e, mb * P:(mb + 1) * P],
                                     x4r[:, :R], start=True, stop=False)
                    nc.tensor.matmul(ps_z[:, mb, :R],
                                     w0b[:, e, mb * P:(mb + 1) * P],
                                     xe[:, :R], start=False, stop=True)
                h = hpool.tile([P, 2, 512], F32R, tag="h")
                nc.scalar.activation(h[:, 0, :R], ps_z[:, 0, :R],
                                     mybir.ActivationFunctionType.Relu,
                                     bias=b0[:, e, 0:1], scale=1.0)
                nc.scalar.activation(h[:, 1, :R], ps_z[:, 1, :R],
                                     mybir.ActivationFunctionType.Relu,
                                     bias=b0[:, e, 1:2], scale=1.0)

                # hidden residual layers (third residual folded into the
                # output layer via the s3-prescaled Wout)
                t3 = None
                for k in range(NL - 1):
                    ps_zk = psB.tile([P, 2, 512], F32, tag="z")
                    for mb in range(2):
                        for kb in range(2):
                            nc.tensor.matmul(
                                ps_zk[:, mb, :R],
                                wh[:, e, k, kb, mb * P:(mb + 1) * P],
                                h[:, kb, :R],
                                start=(kb == 0), stop=(kb == 1))
                    t = hpool.tile([P, 2, 512], F32R, tag="t")
                    nc.scalar.activation(t[:, 0, :R], ps_zk[:, 0, :R],
                                         mybir.ActivationFunctionType.Relu,
                                         bias=bh[:, e, k, 0:1], scale=1.0)
                    if k == 2:
                        nc.scalar.activation(t[:, 1, :R], ps_zk[:, 1, :R],
                                             mybir.ActivationFunctionType.Relu,
                                             bias=bh[:, e, k, 1:2], scale=1.0)
                    else:
                        nc.vector.tensor_scalar(t[:, 1, :R], ps_zk[:, 1, :R],
                                                bh[:, e, k, 1:2], 0.0,
                                                mybir.AluOpType.add,
                                                mybir.AluOpType.max)
                    if k == 2:
                        t3 = t
                        break
                    h_new = hpool.tile([P, 2, 512], F32R, tag="h")
                    idx = e * (NL - 1) + k
                    nc.vector.scalar_tensor_tensor(
                        h_new[:, :, :R].rearrange("p b r -> p (b r)") if R == 512
                        else h_new[:, :, :R],
                        t[:, :, :R].rearrange("p b r -> p (b r)") if R == 512
                        else t[:, :, :R],
                        scl[:, idx:idx + 1],
                        h[:, :, :R].rearrange("p b r -> p (b r)") if R == 512
                        else h[:, :, :R],
                        mybir.AluOpType.mult, mybir.AluOpType.add)
                    h = h_new

                # output layer: o = Wout^T h2 + (s3 Wout)^T t3 + bout
                ps_o = psA.tile([DOUT, 512], F32, tag="x4o")
                for kb in range(2):
                    nc.tensor.matmul(ps_o[:, :R], wo[:, e, kb, :], h[:, kb, :R],
                                     start=(kb == 0), stop=False)
                for kb in range(2):
                    nc.tensor.matmul(ps_o[:, :R], wos[:, e, kb, :],
                                     t3[:, kb, :R],
                                     start=False, stop=(kb == 1))
                oT = wpool.tile([DOUT, 512], F32, tag="oT")
                nc.scalar.activation(oT[:, :R], ps_o[:, :R],
                                     mybir.ActivationFunctionType.Identity,
                                     bias=bo[:, e:e + 1], scale=1.0)

                # transpose back to rows, divide by in_dim, store
                ps_t = psA.tile([P, 4, DOUT], F32, tag="t")
                for ch in range(c):
                    nc.tensor.transpose(ps_t[:, ch, :],
                                        oT[:, ch * P:(ch + 1) * P],
                                        ident[:DOUT, :DOUT])
                rid = wpool.tile([P, 4], F32, tag="rid")
                nc.vector.reciprocal(rid[:, :c], d_t[:, :c])
                o_rows = wpool.tile([P, 4, DOUT], F32, tag="o_rows")
                nc.vector.tensor_mul(
                    o_rows[:, :c, :], ps_t[:, :c, :],
                    rid[:, :c, None].to_broadcast((P, c, DOUT)))
                nc.sync.dma_start(
                    out=bass.AP(tensor=out_d.tensor, offset=r0 * DOUT,
                                ap=[[DOUT, P], [P * DOUT, c], [1, DOUT]]),
                    in_=o_rows[:, :c, :])

            TILES = [512, 512, 512, 512, 256]
            for e in range(E):
                r0 = e * CAP
                for R in TILES:
                    do_tile(e, r0, R)
                    r0 += R

    nc.compile()
    return nc


def _get_program():
    if "nc" not in _compiled:
        _compiled["nc"] = _build_program()
    return _compiled["nc"]


def _prep_weights(W0, b0, Wh, bh, scal, Wout, bout):
    """Host-side layout transforms (permutation / reshape / replication only)."""
    # xe feature order on device: p = s*40 + j*10 + i  (s: 0=sin 1=cos)
    # reference xe column order: 4 + i*8 + j*2 + s
    # Bsel rows 0..3 select dim j scaled by freq/2pi = 2^(i-1) (exact);
    # row 4 (against the ones input row) adds 0.25 turn on cos rows.
    Bsel = np.zeros((5, 80), np.float32)
    perm = np.zeros(80, np.int64)
    for s in range(2):
        for j in range(4):
            for i in range(NUM_FREQS):
                p = s * 40 + j * 10 + i
                Bsel[j, p] = np.float32(2.0 ** (i - 1))
                Bsel[4, p] = 0.0 if s == 0 else 0.25
                perm[p] = 4 + i * 8 + j * 2 + s
    w0a = np.ascontiguousarray(W0[:, :4, :].transpose(1, 0, 2))      # [4,E,H]
    w0b = np.ascontiguousarray(W0[:, perm, :].transpose(1, 0, 2))    # [80,E,H]
    wh = np.ascontiguousarray(
        Wh.reshape(E, NL - 1, 2, 128, HID).transpose(3, 0, 1, 2, 4))  # [128,E,3,2,H]
    wo = np.ascontiguousarray(
        Wout.reshape(E, 2, 128, DOUT).transpose(2, 0, 1, 3))          # [128,E,2,Do]
    b0r = np.ascontiguousarray(b0.reshape(E, 2, 128).transpose(2, 0, 1))
    bhr = np.ascontiguousarray(
        bh.reshape(E, NL - 1, 2, 128).transpose(3, 0, 1, 2))
    bor = np.ascontiguousarray(bout.transpose(1, 0))                  # [Do,E]
    sc12 = np.ascontiguousarray(scal.reshape(-1))
    return dict(bsel=Bsel, w0a=w0a, w0b=w0b, wh=wh, wo=wo,
                b0r=b0r, bhr=bhr, bor=bor, scal12=sc12)


def kernel(x, in_dim, layer_id, W0, b0, Wh, bh, scal, Wout, bout):
    from concourse.bass_utils import run_bass_kernel_spmd

    x = np.asarray(x, np.float32)
    in_dim = np.asarray(in_dim, np.float32)
    layer_id = np.asarray(layer_id)

    # ---- dispatch: per-expert row indices, padded to CAP per core ----
    PADIDX = N
    x_aug = np.vstack([x, np.ones((1, 4), np.float32)])
    d_aug = np.concatenate([in_dim, np.ones(1, np.float32)])
    perms = np.full((NCORE, ROWS_CORE), PADIDX, np.int64)
    overflow = []
    for e in range(E):
        idx = np.flatnonzero(layer_id == e)
        if len(idx) > NCORE * CAP:
            overflow.append(idx[NCORE * CAP:])
            idx = idx[:NCORE * CAP]
        nfull = len(idx) // CAP
        for c in range(nfull):
            perms[c, e * CAP:(e + 1) * CAP] = idx[c * CAP:(c + 1) * CAP]
        if nfull < NCORE:
            rem = idx[nfull * CAP:]
            perms[nfull, e * CAP:e * CAP + len(rem)] = rem

    wmaps = _prep_weights(np.asarray(W0, np.float32), np.asarray(b0, np.float32),
                          np.asarray(Wh, np.float32), np.asarray(bh, np.float32),
                          np.asarray(scal, np.float32),
                          np.asarray(Wout, np.float32),
                          np.asarray(bout, np.float32))

    in_maps = []
    for c in range(NCORE):
        p = perms[c]
        m = dict(wmaps)
        m["x_rows"] = np.ascontiguousarray(x_aug[p])
        m["indim_rows"] = np.ascontiguousarray(d_aug[p])
        in_maps.append(m)

    nc = _get_program()
    res = run_bass_kernel_spmd(nc, in_maps, core_ids=list(range(NCORE)),
                               **RUN_KWARGS)
    LAST_RESULT.clear()
    LAST_RESULT.append(res)

    out = np.zeros((N + 1, DOUT), np.float32)
    for c in range(NCORE):
        out[perms[c]] = res.results[c]["out_rows"]

    # pathological overflow fallback (never hit for the benchmark input)
    if overflow:
        ov = np.concatenate(overflow)
        out[ov] = _numpy_ref(x[ov], in_dim[ov], layer_id[ov], W0, b0, Wh, bh,
                             scal, Wout, bout)
    return out[:N]


def _numpy_ref(x, in_dim, layer_id, W0, b0, Wh, bh, scal, Wout, bout):
    x = np.concatenate([x[:, :3] / x[:, 3:4], x[:, 3:]], axis=1)
    freqs = _freqs_f32()
    ang = x[:, None, :] * freqs[None, :, None]
    sc = np.stack([np.sin(ang), np.cos(ang)], axis=-1)
    xe = np.concatenate([x, sc.reshape(x.shape[0], -1)], axis=1)
    out = np.zeros((x.shape[0], DOUT), np.float32)
    for e in range(E):
        m = layer_id == e
        if not m.any():
            continue
        h = np.maximum(xe[m] @ W0[e] + b0[e], 0.0)
        for k in range(NL - 1):
            h = scal[e, k] * np.maximum(h @ Wh[e, k] + bh[e, k], 0.0) + h
        out[m] = h @ Wout[e] + bout[e]
    return out / in_dim[:, None]



# revision 18
# speedup vs baseline: 1.5630x; 1.5630x over previous
"""Trainium2 Bass kernel for nn_NeRF_MLP_Compose (MoE-routed NeRF MLP).

Strategy (v2):
  - Host-side MoE dispatch: each expert's rows are split across a PAIR of
    cores (core c handles expert c//2), so each core runs ONE expert dense
    over ~8.2k rows (CAP=8704 padded) and holds only that expert's weights.
  - bf16 weights + activations for all matmuls (tolerance is 2e-2); the
    positional-encoding angle path stays fp32 for phase accuracy.
  - Row-major front-end: normalize + angle/[mod 1]/sin/cos are computed with
    rows on partitions (no PE transposes, no PSUM copies); the encoded
    features are flipped to feature-major with the DMA XBAR transpose
    (16-bit, 16x128 tiles).  Output is flipped back the same way.
  - MLP: feature-major, K<=128 stationary blocks, N=512 moving tiles.
    Third residual folded into the out layer input (h3 = s2*t3 + h2).
  - Element-wise work is spread across ACT / DVE / GPSIMD so each engine's
    per-tile time roughly matches the PE's; the Tile list-scheduler
    overlaps tiles (all pools are multi-buffered).
"""
import sys
for _p in ("/opt/trn_rl_repo", "/root/.axon_site/_ro/trn_rl_repo"):
    if _p not in sys.path:
        sys.path.insert(0, _p)

import numpy as np
import ml_dtypes

N = 65536
E = 4            # experts
NCORE = 8
CAP = 8704       # rows per core (one expert per core pair; 2*CAP=17408 >> E[16384])
NT = 17          # 512-row tiles per core
R = 512          # rows per tile
C = 4            # 128-row chunks per tile
NUM_FREQS = 10
HID = 256
DOUT = 64
NL = 4           # layers -> 3 residual blocks
TWO_PI_F32 = float(np.float32(2 * np.pi))
HALF_PI_F32 = float(np.float32(0.5 * np.pi))
MAGIC_C = float(np.float32(1.5 * 2 ** 23))

_compiled = {}
RUN_KWARGS = {}    # test.py may set e.g. {"trace": True}
LAST_RESULT = []   # test.py reads the BassKernelResults appended here


def _build_program():
    import concourse.bass as bass
    from concourse import bacc
    import concourse.mybir as mybir
    import concourse.tile as tile

    F32 = mybir.dt.float32
    BF16 = mybir.dt.bfloat16
    P = 128
    ALU = mybir.AluOpType
    ACTF = mybir.ActivationFunctionType

    nc = bacc.Bacc("TRN2", target_bir_lowering=False, debug=False)

    # ---- DRAM I/O (per core; one expert's weights) ----
    x_d = nc.dram_tensor("x_rows", [CAP, 4], F32, kind="ExternalInput").ap()
    d_d = nc.dram_tensor("indim_rows", [CAP], F32, kind="ExternalInput").ap()
    fr_d = nc.dram_tensor("fr10", [NUM_FREQS], F32, kind="ExternalInput").ap()
    w0_d = nc.dram_tensor("w0", [85, HID], BF16, kind="ExternalInput").ap()
    wh_d = nc.dram_tensor("wh", [P, NL - 1, 2, 2, P], BF16,
                          kind="ExternalInput").ap()
    wo_d = nc.dram_tensor("wo", [P, 2, DOUT], BF16, kind="ExternalInput").ap()
    b0_d = nc.dram_tensor("b0r", [P, 2], F32, kind="ExternalInput").ap()
    bh_d = nc.dram_tensor("bhr", [P, NL - 1, 2], F32, kind="ExternalInput").ap()
    bo_d = nc.dram_tensor("bor", [DOUT, 1], F32, kind="ExternalInput").ap()
    sc_d = nc.dram_tensor("scal3", [NL - 1], F32, kind="ExternalInput").ap()
    out_d = nc.dram_tensor("out_rows", [CAP, DOUT], F32,
                           kind="ExternalOutput").ap()

    with tile.TileContext(nc) as tc:
        with tc.tile_pool(name="const", bufs=1) as cpool, \
             tc.tile_pool(name="fr", bufs=3) as fpool, \
             tc.tile_pool(name="act", bufs=8) as apool, \
             tc.tile_pool(name="psz", bufs=3, space="PSUM") as zpool, \
             tc.tile_pool(name="pso", bufs=2, space="PSUM") as opool:

            # ---- constants / weights into SBUF (once) ----
            w0 = cpool.tile([85, HID], BF16)
            nc.gpsimd.dma_start(out=w0, in_=w0_d)
            wh = cpool.tile([P, NL - 1, 2, 2, P], BF16)
            nc.gpsimd.dma_start(out=wh, in_=wh_d)
            wo = cpool.tile([P, 2, DOUT], BF16)
            nc.gpsimd.dma_start(out=wo, in_=wo_d)
            b0 = cpool.tile([P, 2], F32)
            nc.gpsimd.dma_start(out=b0, in_=b0_d)
            bh = cpool.tile([P, NL - 1, 2], F32)
            nc.gpsimd.dma_start(out=bh, in_=bh_d)
            bo = cpool.tile([DOUT, 1], F32)
            nc.gpsimd.dma_start(out=bo, in_=bo_d)
            scl = cpool.tile([P, NL - 1], F32)
            nc.gpsimd.dma_start(
                out=scl,
                in_=bass.AP(tensor=sc_d.tensor, offset=0,
                            ap=[[0, P], [1, NL - 1]]))
            fr = cpool.tile([P, NUM_FREQS], F32)
            nc.gpsimd.dma_start(
                out=fr,
                in_=bass.AP(tensor=fr_d.tensor, offset=0,
                            ap=[[0, P], [1, NUM_FREQS]]))
            ph = cpool.tile([P, 2], F32)
            nc.vector.memset(ph[:, 0:1], 0.0)
            nc.vector.memset(ph[:, 1:2], 0.25)

            def do_tile(t):
                r0 = t * R
                # ---- row-major front-end ----
                x_t = fpool.tile([P, C, 4], F32, tag="x_t")
                nc.sync.dma_start(
                    out=x_t,
                    in_=bass.AP(tensor=x_d.tensor, offset=r0 * 4,
                                ap=[[4, P], [4 * P, C], [1, 4]]))
                d_t = fpool.tile([P, C], F32, tag="d_t")
                nc.sync.dma_start(
                    out=d_t,
                    in_=bass.AP(tensor=d_d.tensor, offset=r0,
                                ap=[[1, P], [P, C]]))

                rc = fpool.tile([P, C], F32, tag="rc")
                nc.vector.reciprocal(rc, x_t[:, :, 3])
                xn = fpool.tile([P, C, 4], F32, tag="xn")
                nc.gpsimd.tensor_mul(xn, x_t,
                                     rc[:, :, None].to_broadcast((P, C, 4)))
                nc.gpsimd.tensor_copy(xn[:, :, 3], x_t[:, :, 3])

                # angles in turns: t20[p, c, j, i] = x'_j * 2^(i-1)  (exact);
                # t40 doubles it with the cos quarter-turn phase (folded in
                # BEFORE range reduction -- the Sin table domain is ~[-pi,pi])
                t20 = fpool.tile([P, C, 4, NUM_FREQS], F32, tag="t20")
                nc.gpsimd.tensor_mul(
                    t20, xn[:, :, :, None].to_broadcast((P, C, 4, NUM_FREQS)),
                    fr[:, None, None, :].to_broadcast((P, C, 4, NUM_FREQS)))
                t20f = t20.rearrange("p c j i -> p c (j i)")
                t40 = fpool.tile([P, C, 2, 40], F32, tag="t40")
                nc.gpsimd.tensor_tensor(
                    t40,
                    t20f[:, :, None, :].to_broadcast((P, C, 2, 40)),
                    ph[:, None, :, None].to_broadcast((P, C, 2, 40)),
                    ALU.add)
                # k = round(t40) via fp32 magic add; m40 = t40 - k in [-.5,.5]
                kt = fpool.tile([P, C, 2, 40], F32, tag="kt")
                nc.vector.tensor_scalar(kt, t40, MAGIC_C, MAGIC_C,
                                        ALU.add, ALU.subtract)
                m40 = fpool.tile([P, C, 2, 40], F32, tag="m40")
                nc.vector.scalar_tensor_tensor(m40, kt, -1.0, t40,
                                               ALU.mult, ALU.add)

                # xe rows: [0:4]=x', [4:44]=sin, [44:84]=cos, [84]=1 (bias
                # row for the l0 matmul), [85:128]=junk
                xe_r = fpool.tile([P, C, P], BF16, tag="xe_r")
                nc.gpsimd.tensor_copy(xe_r[:, :, 0:4], xn)
                nc.gpsimd.memset(xe_r[:, :, 84:85], 1.0)
                m40f = m40.rearrange("p c s f -> p c (s f)")
                nc.scalar.activation(xe_r[:, :, 4:84], m40f, ACTF.Sin,
                                     bias=0.0, scale=TWO_PI_F32)

                # flip to feature-major via DMA XBAR transpose
                xe = apool.tile([P, R], BF16, tag="xe")
                nc.sync.dma_start(out=xe.rearrange("p (c q) -> p c q", c=C),
                                  in_=xe_r, transpose=True)

                # ---- layer 0: h0 = relu(W0^T xe)  (bias via the ones row,
                # so one biasless relu covers both halves) ----
                h = apool.tile([P, 2, R], BF16, tag="h")
                z0 = zpool.tile([P, 2, R], F32, tag="z")
                nc.tensor.matmul(z0[:, 0, :], w0[:, 0:P], xe[0:85, :],
                                 start=True, stop=True)
                nc.tensor.matmul(z0[:, 1, :], w0[:, P:HID], xe[0:85, :],
                                 start=True, stop=True)
                nc.scalar.activation(h, z0, ACTF.Relu, bias=0.0, scale=1.0)

                # ---- hidden residual layers ----
                for k in range(NL - 1):
                    zk = zpool.tile([P, 2, R], F32, tag="z")
                    za, zb = zk[:, 0, :], zk[:, 1, :]
                    nc.tensor.matmul(za, wh[:, k, 0, 0, :], h[:, 0, :],
                                     start=True, stop=False)
                    nc.tensor.matmul(za, wh[:, k, 1, 0, :], h[:, 1, :],
                                     start=False, stop=True)
                    nc.tensor.matmul(zb, wh[:, k, 0, 1, :], h[:, 0, :],
                                     start=True, stop=False)
                    nc.tensor.matmul(zb, wh[:, k, 1, 1, :], h[:, 1, :],
                                     start=False, stop=True)
                    tt = apool.tile([P, 2, R], BF16, tag="t")
                    nc.scalar.activation(tt[:, 0, :], za, ACTF.Relu,
                                         bias=bh[:, k, 0:1], scale=1.0)
                    nc.vector.tensor_scalar(tt[:, 1, :], zb, bh[:, k, 1:2],
                                            0.0, ALU.add, ALU.max)
                    # h_{k+1} = s_k * t + h_k   (k==2 folds the third
                    # residual into the out-layer input)
                    h_new = apool.tile([P, 2, R], BF16, tag="h")
                    nc.vector.scalar_tensor_tensor(h_new, tt, scl[:, k:k + 1],
                                                   h, ALU.mult, ALU.add)
                    h = h_new

                # ---- out layer: o = Wout^T h3 + bout ----
                o_ps = opool.tile([DOUT, R], F32, tag="o")
                nc.tensor.matmul(o_ps, wo[:, 0, :], h[:, 0, :],
                                 start=True, stop=False)
                nc.tensor.matmul(o_ps, wo[:, 1, :], h[:, 1, :],
                                 start=False, stop=True)
                oT = fpool.tile([DOUT, R], BF16, tag="oT")
                nc.scalar.activation(oT, o_ps, ACTF.Identity,
                                     bias=bo, scale=1.0)

                # flip back to row-major, divide by in_dim, store
                o_r = fpool.tile([P, C, DOUT], BF16, tag="o_r")
                nc.sync.dma_start(out=o_r, in_=oT, transpose=True)
                rid = fpool.tile([P, C], F32, tag="rid")
                nc.vector.reciprocal(rid, d_t)
                o_f = fpool.tile([P, C, DOUT], F32, tag="o_f")
                nc.gpsimd.tensor_mul(
                    o_f, o_r, rid[:, :, None].to_broadcast((P, C, DOUT)))
                nc.sync.dma_start(
                    out=bass.AP(tensor=out_d.tensor, offset=r0 * DOUT,
                                ap=[[DOUT, P], [P * DOUT, C], [1, DOUT]]),
                    in_=o_f)

            for t in range(NT):
                do_tile(t)

    nc.compile()
    return nc


def _get_program():
    if "nc" not in _compiled:
        _compiled["nc"] = _build_program()
    return _compiled["nc"]


def _xe_perm():
    """perm[slot] = reference xe column for device slot order
    (slots: 0..3 = x', 4 + j*10 + i = sin, 44 + j*10 + i = cos)."""
    perm = np.zeros(84, np.int64)
    perm[0:4] = np.arange(4)
    for s in range(2):
        for j in range(4):
            for i in range(NUM_FREQS):
                perm[4 + s * 40 + j * 10 + i] = 4 + i * 8 + j * 2 + s
    return perm


def _prep_weights(e, W0, b0, Wh, bh, scal, Wout, bout):
    """Host-side layout transforms (permutation / reshape / cast only)."""
    bf = ml_dtypes.bfloat16
    w0 = np.ascontiguousarray(
        np.vstack([W0[e][_xe_perm()], b0[e][None, :]])).astype(bf)  # [85,256]
    wh = np.ascontiguousarray(
        Wh[e].reshape(NL - 1, 2, 128, 2, 128)
        .transpose(2, 0, 1, 3, 4)).astype(bf)                      # [128,3,2,2,128]
    wo = np.ascontiguousarray(
        Wout[e].reshape(2, 128, DOUT).transpose(1, 0, 2)).astype(bf)
    b0r = np.ascontiguousarray(b0[e].reshape(2, 128).T)            # [128,2]
    bhr = np.ascontiguousarray(
        bh[e].reshape(NL - 1, 2, 128).transpose(2, 0, 1))          # [128,3,2]
    bor = np.ascontiguousarray(bout[e].reshape(DOUT, 1))
    sc3 = np.ascontiguousarray(scal[e])
    fr10 = (2.0 ** (np.arange(NUM_FREQS, dtype=np.float32) - 1.0)).astype(
        np.float32)
    return dict(w0=w0, wh=wh, wo=wo, b0r=b0r, bhr=bhr, bor=bor,
                scal3=sc3, fr10=fr10)


def kernel(x, in_dim, layer_id, W0, b0, Wh, bh, scal, Wout, bout):
    from concourse.bass_utils import run_bass_kernel_spmd

    x = np.asarray(x, np.float32)
    in_dim = np.asarray(in_dim, np.float32)
    layer_id = np.asarray(layer_id)
    W0 = np.asarray(W0, np.float32)
    b0 = np.asarray(b0, np.float32)
    Wh = np.asarray(Wh, np.float32)
    bh = np.asarray(bh, np.float32)
    scal = np.asarray(scal, np.float32)
    Wout = np.asarray(Wout, np.float32)
    bout = np.asarray(bout, np.float32)

    # ---- dispatch: expert e -> cores 2e, 2e+1; pad to CAP per core ----
    PADIDX = N
    x_aug = np.vstack([x, np.ones((1, 4), np.float32)])
    d_aug = np.concatenate([in_dim, np.ones(1, np.float32)])
    perms = np.full((NCORE, CAP), PADIDX, np.int64)
    overflow = []
    for e in range(E):
        idx = np.flatnonzero(layer_id == e)
        if len(idx) > 2 * CAP:
            overflow.append(idx[2 * CAP:])
            idx = idx[:2 * CAP]
        nh = min((len(idx) + 1) // 2, CAP)
        perms[2 * e, :nh] = idx[:nh]
        perms[2 * e + 1, :len(idx) - nh] = idx[nh:]

    in_maps = []
    for c in range(NCORE):
        m = _prep_weights(c // 2, W0, b0, Wh, bh, scal, Wout, bout)
        p = perms[c]
        m["x_rows"] = np.ascontiguousarray(x_aug[p])
        m["indim_rows"] = np.ascontiguousarray(d_aug[p])
        in_maps.append(m)

    nc = _get_program()
    res = run_bass_kernel_spmd(nc, in_maps, core_ids=list(range(NCORE)),
                               **RUN_KWARGS)
    LAST_RESULT.clear()
    LAST_RESULT.append(res)

    out = np.zeros((N + 1, DOUT), np.float32)
    for c in range(NCORE):
        out[perms[c]] = np.asarray(res.results[c]["out_rows"], np.float32)

    # pathological overflow fallback (never hit for the benchmark input)
    if overflow:
        ov = np.concatenate(overflow)
        out[ov] = _numpy_ref(x[ov], in_dim[ov], layer_id[ov], W0, b0, Wh, bh,
                             scal, Wout, bout)
    return out[:N]


def _numpy_ref(x, in_dim, layer_id, W0, b0, Wh, bh, scal, Wout, bout):
    x = np.concatenate([x[:, :3] / x[:, 3:4], x[:, 3:]], axis=1)
    freqs = (2.0 ** np.arange(NUM_FREQS, dtype=np.float32)) * np.float32(np.pi)
    ang = x[:, None, :] * freqs[None, :, None]
    sc = np.stack([np.sin(ang), np.cos(ang)], axis=-1)
    xe = np.concatenate([x, sc.reshape(x.shape[0], -1)], axis=1)
    out = np.zeros((x.shape[0], DOUT), np.float32)
    for e in range(E):
        m = layer_id == e
        if not m.any():
            continue
        h = np.maximum(xe[m] @ W0[e] + b0[e], 0.0)
        for k in range(NL - 1):
            h = scal[e, k] * np.maximum(h @ Wh[e, k] + bh[e, k], 0.0) + h
        out[m] = h @ Wout[e] + bout[e]
    return out / in_dim[:, None]


# revision 21
# speedup vs baseline: 2.2737x; 1.4547x over previous
"""Trainium2 Bass kernel for nn_NeRF_MLP_Compose (MoE-routed NeRF MLP).

Strategy (v2):
  - Host-side MoE dispatch: each expert's rows are split across a PAIR of
    cores (core c handles expert c//2), so each core runs ONE expert dense
    over ~8.2k rows (CAP=8704 padded) and holds only that expert's weights.
  - bf16 weights + activations for all matmuls (tolerance is 2e-2); the
    positional-encoding angle path stays fp32 for phase accuracy.
  - Row-major front-end: normalize + angle/[mod 1]/sin/cos are computed with
    rows on partitions (no PE transposes, no PSUM copies); the encoded
    features are flipped to feature-major with the DMA XBAR transpose
    (16-bit, 16x128 tiles).  Output is flipped back the same way.
  - MLP: feature-major, K<=128 stationary blocks, N=512 moving tiles.
    Third residual folded into the out layer input (h3 = s2*t3 + h2).
  - Element-wise work is spread across ACT / DVE / GPSIMD so each engine's
    per-tile time roughly matches the PE's; the Tile list-scheduler
    overlaps tiles (all pools are multi-buffered).
"""
import sys
for _p in ("/opt/trn_rl_repo", "/root/.axon_site/_ro/trn_rl_repo"):
    if _p not in sys.path:
        sys.path.insert(0, _p)

import numpy as np
import ml_dtypes

N = 65536
E = 4            # experts
NCORE = 8
CAP = 8704       # rows per core (one expert per core pair; 2*CAP=17408 >> E[16384])
NT = 17          # 512-row tiles per core
R = 512          # rows per tile
C = 4            # 128-row chunks per tile
NUM_FREQS = 10
HID = 256
DOUT = 64
NL = 4           # layers -> 3 residual blocks
TWO_PI_F32 = float(np.float32(2 * np.pi))
HALF_PI_F32 = float(np.float32(0.5 * np.pi))
MAGIC_C = float(np.float32(1.5 * 2 ** 23))

_compiled = {}
RUN_KWARGS = {}    # test.py may set e.g. {"trace": True}
LAST_RESULT = []   # test.py reads the BassKernelResults appended here


def _build_program():
    import concourse.bass as bass
    from concourse import bacc
    import concourse.mybir as mybir
    import concourse.tile as tile

    F32 = mybir.dt.float32
    BF16 = mybir.dt.bfloat16
    P = 128
    ALU = mybir.AluOpType
    ACTF = mybir.ActivationFunctionType

    nc = bacc.Bacc("TRN2", target_bir_lowering=False, debug=False)

    # ---- DRAM I/O (per core; one expert's weights) ----
    x_d = nc.dram_tensor("x_rows", [CAP, 4], F32, kind="ExternalInput").ap()
    d_d = nc.dram_tensor("indim_rows", [CAP], F32, kind="ExternalInput").ap()
    fr_d = nc.dram_tensor("fr10", [NUM_FREQS], F32, kind="ExternalInput").ap()
    w0_d = nc.dram_tensor("w0", [85, HID], BF16, kind="ExternalInput").ap()
    wh_d = nc.dram_tensor("wh", [P, NL - 1, 2, 2, P], BF16,
                          kind="ExternalInput").ap()
    wo_d = nc.dram_tensor("wo", [P, 2, DOUT], BF16, kind="ExternalInput").ap()
    b0_d = nc.dram_tensor("b0r", [P, 2], F32, kind="ExternalInput").ap()
    bh_d = nc.dram_tensor("bhr", [P, NL - 1, 2], F32, kind="ExternalInput").ap()
    bo_d = nc.dram_tensor("bor", [DOUT, 1], F32, kind="ExternalInput").ap()
    sc_d = nc.dram_tensor("scal3", [NL - 1], F32, kind="ExternalInput").ap()
    out_d = nc.dram_tensor("out_rows", [CAP, DOUT], F32,
                           kind="ExternalOutput").ap()

    with tile.TileContext(nc) as tc:
        with tc.tile_pool(name="const", bufs=1) as cpool, \
             tc.tile_pool(name="fr", bufs=4) as fpool, \
             tc.tile_pool(name="act", bufs=8) as apool, \
             tc.tile_pool(name="psz", bufs=3, space="PSUM") as zpool, \
             tc.tile_pool(name="pso", bufs=2, space="PSUM") as opool:

            # ---- constants / weights into SBUF (once) ----
            w0 = cpool.tile([85, HID], BF16)
            nc.gpsimd.dma_start(out=w0, in_=w0_d)
            wh = cpool.tile([P, NL - 1, 2, 2, P], BF16)
            nc.gpsimd.dma_start(out=wh, in_=wh_d)
            wo = cpool.tile([P, 2, DOUT], BF16)
            nc.gpsimd.dma_start(out=wo, in_=wo_d)
            b0 = cpool.tile([P, 2], F32)
            nc.gpsimd.dma_start(out=b0, in_=b0_d)
            bh = cpool.tile([P, NL - 1, 2], F32)
            nc.gpsimd.dma_start(out=bh, in_=bh_d)
            bo = cpool.tile([DOUT, 1], F32)
            nc.gpsimd.dma_start(out=bo, in_=bo_d)
            scl = cpool.tile([P, NL - 1], F32)
            nc.gpsimd.dma_start(
                out=scl,
                in_=bass.AP(tensor=sc_d.tensor, offset=0,
                            ap=[[0, P], [1, NL - 1]]))
            fr = cpool.tile([P, NUM_FREQS], F32)
            nc.gpsimd.dma_start(
                out=fr,
                in_=bass.AP(tensor=fr_d.tensor, offset=0,
                            ap=[[0, P], [1, NUM_FREQS]]))
            ph = cpool.tile([P, 2], F32)
            nc.vector.memset(ph[:, 0:1], 0.0)
            nc.vector.memset(ph[:, 1:2], 0.25)
            # s2-prescaled out weights: out = Wo^T h2 + (s2 Wo)^T t3, which
            # removes the third residual STT from the per-tile loop
            wos = cpool.tile([P, 2, DOUT], BF16)
            nc.vector.tensor_scalar_mul(wos, wo, scl[:, 2:3])

            def front(t):
                """Row-major front-end: load, normalize, encode -> xe."""
                r0 = t * R
                x_t = fpool.tile([P, C, 4], F32, tag="x_t")
                nc.sync.dma_start(
                    out=x_t,
                    in_=bass.AP(tensor=x_d.tensor, offset=r0 * 4,
                                ap=[[4, P], [4 * P, C], [1, 4]]))
                d_t = fpool.tile([P, C], F32, tag="d_t")
                nc.sync.dma_start(
                    out=d_t,
                    in_=bass.AP(tensor=d_d.tensor, offset=r0,
                                ap=[[1, P], [P, C]]))

                rc = fpool.tile([P, C], F32, tag="rc")
                nc.vector.reciprocal(rc, x_t[:, :, 3])
                xn = fpool.tile([P, C, 4], F32, tag="xn")
                nc.gpsimd.tensor_mul(xn, x_t,
                                     rc[:, :, None].to_broadcast((P, C, 4)))
                nc.gpsimd.tensor_copy(xn[:, :, 3], x_t[:, :, 3])

                # angles in turns: t20[p, c, j, i] = x'_j * 2^(i-1)  (exact);
                # t40 doubles it with the cos quarter-turn phase (folded in
                # BEFORE range reduction -- the Sin table domain is ~[-pi,pi])
                t20 = fpool.tile([P, C, 4, NUM_FREQS], F32, tag="t20")
                nc.gpsimd.tensor_mul(
                    t20, xn[:, :, :, None].to_broadcast((P, C, 4, NUM_FREQS)),
                    fr[:, None, None, :].to_broadcast((P, C, 4, NUM_FREQS)))
                t20f = t20.rearrange("p c j i -> p c (j i)")
                t40 = fpool.tile([P, C, 2, 40], F32, tag="t40")
                nc.gpsimd.tensor_tensor(
                    t40,
                    t20f[:, :, None, :].to_broadcast((P, C, 2, 40)),
                    ph[:, None, :, None].to_broadcast((P, C, 2, 40)),
                    ALU.add)
                # k = round(t40) via fp32 magic add; m40 = t40 - k in [-.5,.5]
                kt = fpool.tile([P, C, 2, 40], F32, tag="kt")
                nc.vector.tensor_scalar(kt, t40, MAGIC_C, MAGIC_C,
                                        ALU.add, ALU.subtract)
                m40 = fpool.tile([P, C, 2, 40], F32, tag="m40")
                nc.gpsimd.tensor_tensor(m40, t40, kt, ALU.subtract)

                # xe rows: [0:4]=x', [4:44]=sin, [44:84]=cos, [84]=1 (bias
                # row for the l0 matmul), [85:128]=junk
                xe_r = fpool.tile([P, C, P], BF16, tag="xe_r")
                nc.gpsimd.tensor_copy(xe_r[:, :, 0:4], xn)
                nc.gpsimd.memset(xe_r[:, :, 84:85], 1.0)
                m40f = m40.rearrange("p c s f -> p c (s f)")
                nc.scalar.activation(xe_r[:, :, 4:84], m40f, ACTF.Sin,
                                     bias=0.0, scale=TWO_PI_F32)

                # flip to feature-major via DMA XBAR transpose
                xe = apool.tile([P, R], BF16, tag="xe")
                nc.sync.dma_start(out=xe.rearrange("p (c q) -> p c q", c=C),
                                  in_=xe_r, transpose=True)
                return xe, d_t

            def l0_mm(xe):
                # bias rides the ones row, so one biasless relu covers both
                # halves of z0
                z0 = zpool.tile([P, 2, R], F32, tag="z")
                nc.tensor.matmul(z0[:, 0, :], w0[:, 0:P], xe[0:85, :],
                                 start=True, stop=True)
                nc.tensor.matmul(z0[:, 1, :], w0[:, P:HID], xe[0:85, :],
                                 start=True, stop=True)
                return z0

            def l0_relu(z0):
                h = apool.tile([P, 2, R], BF16, tag="h")
                nc.scalar.activation(h, z0, ACTF.Relu, bias=0.0, scale=1.0)
                return h

            def layer_mm(k, h):
                zk = zpool.tile([P, 2, R], F32, tag="z")
                for mb in range(2):
                    nc.tensor.matmul(zk[:, mb, :], wh[:, k, 0, mb, :],
                                     h[:, 0, :], start=True, stop=False)
                    nc.tensor.matmul(zk[:, mb, :], wh[:, k, 1, mb, :],
                                     h[:, 1, :], start=False, stop=True)
                return zk

            def layer_post(k, zk, h):
                # t = relu(zk + bh);  k<2: h' = s_k t + h;  k==2: keep t3
                # (its residual is folded into the prescaled out weights)
                tt = apool.tile([P, 2, R], BF16, tag="t")
                nc.scalar.activation(tt[:, 0, :], zk[:, 0, :], ACTF.Relu,
                                     bias=bh[:, k, 0:1], scale=1.0)
                if k == 1:
                    nc.scalar.activation(tt[:, 1, :], zk[:, 1, :], ACTF.Relu,
                                         bias=bh[:, k, 1:2], scale=1.0)
                else:
                    nc.vector.tensor_scalar(tt[:, 1, :], zk[:, 1, :],
                                            bh[:, k, 1:2], 0.0,
                                            ALU.add, ALU.max)
                if k == 2:
                    return h, tt
                h_new = apool.tile([P, 2, R], BF16, tag="h")
                nc.vector.scalar_tensor_tensor(h_new, tt, scl[:, k:k + 1],
                                               h, ALU.mult, ALU.add)
                return h_new, None

            def out_mm(h2, t3):
                o_ps = opool.tile([DOUT, R], F32, tag="o")
                nc.tensor.matmul(o_ps, wo[:, 0, :], h2[:, 0, :],
                                 start=True, stop=False)
                nc.tensor.matmul(o_ps, wo[:, 1, :], h2[:, 1, :],
                                 start=False, stop=False)
                nc.tensor.matmul(o_ps, wos[:, 0, :], t3[:, 0, :],
                                 start=False, stop=False)
                nc.tensor.matmul(o_ps, wos[:, 1, :], t3[:, 1, :],
                                 start=False, stop=True)
                return o_ps

            def epilogue(t, o_ps, d_t):
                r0 = t * R
                oT = fpool.tile([DOUT, R], BF16, tag="oT")
                nc.scalar.activation(oT, o_ps, ACTF.Identity,
                                     bias=bo, scale=1.0)
                # flip back to row-major, divide by in_dim, store
                o_r = fpool.tile([P, C, DOUT], BF16, tag="o_r")
                nc.scalar.dma_start(out=o_r, in_=oT, transpose=True)
                rid = fpool.tile([P, C], F32, tag="rid")
                nc.vector.reciprocal(rid, d_t)
                o_f = fpool.tile([P, C, DOUT], F32, tag="o_f")
                nc.gpsimd.tensor_mul(
                    o_f, o_r, rid[:, :, None].to_broadcast((P, C, DOUT)))
                nc.scalar.dma_start(
                    out=bass.AP(tensor=out_d.tensor, offset=r0 * DOUT,
                                ap=[[DOUT, P], [P * DOUT, C], [1, DOUT]]),
                    in_=o_f)

            # two tiles interleaved per layer so the PE always has a ready
            # matmul burst while the other tile's relu/residual chain runs
            for t0 in range(0, NT, 2):
                pair = [t0] if t0 + 1 >= NT else [t0, t0 + 1]
                st = {}
                for t in pair:
                    st[t] = {}
                    st[t]["xe"], st[t]["d"] = front(t)
                for t in pair:
                    st[t]["z"] = l0_mm(st[t]["xe"])
                for t in pair:
                    st[t]["h"] = l0_relu(st[t]["z"])
                for k in range(NL - 1):
                    for t in pair:
                        st[t]["zk"] = layer_mm(k, st[t]["h"])
                    for t in pair:
                        st[t]["h"], st[t]["t3"] = layer_post(
                            k, st[t]["zk"], st[t]["h"])
                for t in pair:
                    st[t]["o"] = out_mm(st[t]["h"], st[t]["t3"])
                for t in pair:
                    epilogue(t, st[t]["o"], st[t]["d"])

    nc.compile()
    return nc


def _get_program():
    if "nc" not in _compiled:
        _compiled["nc"] = _build_program()
    return _compiled["nc"]


def _xe_perm():
    """perm[slot] = reference xe column for device slot order
    (slots: 0..3 = x', 4 + j*10 + i = sin, 44 + j*10 + i = cos)."""
    perm = np.zeros(84, np.int64)
    perm[0:4] = np.arange(4)
    for s in range(2):
        for j in range(4):
            for i in range(NUM_FREQS):
                perm[4 + s * 40 + j * 10 + i] = 4 + i * 8 + j * 2 + s
    return perm


def _prep_weights(e, W0, b0, Wh, bh, scal, Wout, bout):
    """Host-side layout transforms (permutation / reshape / cast only)."""
    bf = ml_dtypes.bfloat16
    w0 = np.ascontiguousarray(
        np.vstack([W0[e][_xe_perm()], b0[e][None, :]])).astype(bf)  # [85,256]
    wh = np.ascontiguousarray(
        Wh[e].reshape(NL - 1, 2, 128, 2, 128)
        .transpose(2, 0, 1, 3, 4)).astype(bf)                      # [128,3,2,2,128]
    wo = np.ascontiguousarray(
        Wout[e].reshape(2, 128, DOUT).transpose(1, 0, 2)).astype(bf)
    b0r = np.ascontiguousarray(b0[e].reshape(2, 128).T)            # [128,2]
    bhr = np.ascontiguousarray(
        bh[e].reshape(NL - 1, 2, 128).transpose(2, 0, 1))          # [128,3,2]
    bor = np.ascontiguousarray(bout[e].reshape(DOUT, 1))
    sc3 = np.ascontiguousarray(scal[e])
    fr10 = (2.0 ** (np.arange(NUM_FREQS, dtype=np.float32) - 1.0)).astype(
        np.float32)
    return dict(w0=w0, wh=wh, wo=wo, b0r=b0r, bhr=bhr, bor=bor,
                scal3=sc3, fr10=fr10)


def kernel(x, in_dim, layer_id, W0, b0, Wh, bh, scal, Wout, bout):
    from concourse.bass_utils import run_bass_kernel_spmd

    x = np.asarray(x, np.float32)
    in_dim = np.asarray(in_dim, np.float32)
    layer_id = np.asarray(layer_id)
    W0 = np.asarray(W0, np.float32)
    b0 = np.asarray(b0, np.float32)
    Wh = np.asarray(Wh, np.float32)
    bh = np.asarray(bh, np.float32)
    scal = np.asarray(scal, np.float32)
    Wout = np.asarray(Wout, np.float32)
    bout = np.asarray(bout, np.float32)

    # ---- dispatch: expert e -> cores 2e, 2e+1; pad to CAP per core ----
    PADIDX = N
    x_aug = np.vstack([x, np.ones((1, 4), np.float32)])
    d_aug = np.concatenate([in_dim, np.ones(1, np.float32)])
    perms = np.full((NCORE, CAP), PADIDX, np.int64)
    overflow = []
    for e in range(E):
        idx = np.flatnonzero(layer_id == e)
        if len(idx) > 2 * CAP:
            overflow.append(idx[2 * CAP:])
            idx = idx[:2 * CAP]
        nh = min((len(idx) + 1) // 2, CAP)
        perms[2 * e, :nh] = idx[:nh]
        perms[2 * e + 1, :len(idx) - nh] = idx[nh:]

    in_maps = []
    for c in range(NCORE):
        m = _prep_weights(c // 2, W0, b0, Wh, bh, scal, Wout, bout)
        p = perms[c]
        m["x_rows"] = np.ascontiguousarray(x_aug[p])
        m["indim_rows"] = np.ascontiguousarray(d_aug[p])
        in_maps.append(m)

    nc = _get_program()
    res = run_bass_kernel_spmd(nc, in_maps, core_ids=list(range(NCORE)),
                               **RUN_KWARGS)
    LAST_RESULT.clear()
    LAST_RESULT.append(res)

    out = np.zeros((N + 1, DOUT), np.float32)
    for c in range(NCORE):
        out[perms[c]] = np.asarray(res.results[c]["out_rows"], np.float32)

    # pathological overflow fallback (never hit for the benchmark input)
    if overflow:
        ov = np.concatenate(overflow)
        out[ov] = _numpy_ref(x[ov], in_dim[ov], layer_id[ov], W0, b0, Wh, bh,
                             scal, Wout, bout)
    return out[:N]


def _numpy_ref(x, in_dim, layer_id, W0, b0, Wh, bh, scal, Wout, bout):
    x = np.concatenate([x[:, :3] / x[:, 3:4], x[:, 3:]], axis=1)
    freqs = (2.0 ** np.arange(NUM_FREQS, dtype=np.float32)) * np.float32(np.pi)
    ang = x[:, None, :] * freqs[None, :, None]
    sc = np.stack([np.sin(ang), np.cos(ang)], axis=-1)
    xe = np.concatenate([x, sc.reshape(x.shape[0], -1)], axis=1)
    out = np.zeros((x.shape[0], DOUT), np.float32)
    for e in range(E):
        m = layer_id == e
        if not m.any():
            continue
        h = np.maximum(xe[m] @ W0[e] + b0[e], 0.0)
        for k in range(NL - 1):
            h = scal[e, k] * np.maximum(h @ Wh[e, k] + bh[e, k], 0.0) + h
        out[m] = h @ Wout[e] + bout[e]
    return out / in_dim[:, None]


# revision 24
# speedup vs baseline: 2.4641x; 1.0837x over previous
"""Trainium2 Bass kernel for nn_NeRF_MLP_Compose (MoE-routed NeRF MLP).

Strategy (v2):
  - Host-side MoE dispatch: each expert's rows are split across a PAIR of
    cores (core c handles expert c//2), so each core runs ONE expert dense
    over ~8.2k rows (CAP=8704 padded) and holds only that expert's weights.
  - bf16 weights + activations for all matmuls (tolerance is 2e-2); the
    positional-encoding angle path stays fp32 for phase accuracy.
  - Row-major front-end: normalize + angle/[mod 1]/sin/cos are computed with
    rows on partitions (no PE transposes, no PSUM copies); the encoded
    features are flipped to feature-major with the DMA XBAR transpose
    (16-bit, 16x128 tiles).  Output is flipped back the same way.
  - MLP: feature-major, K<=128 stationary blocks, N=512 moving tiles.
    Third residual folded into the out layer input (h3 = s2*t3 + h2).
  - Element-wise work is spread across ACT / DVE / GPSIMD so each engine's
    per-tile time roughly matches the PE's; the Tile list-scheduler
    overlaps tiles (all pools are multi-buffered).
"""
import sys
for _p in ("/opt/trn_rl_repo", "/root/.axon_site/_ro/trn_rl_repo"):
    if _p not in sys.path:
        sys.path.insert(0, _p)

import numpy as np
import ml_dtypes

N = 65536
E = 4            # experts
NCORE = 8
CAP = 8704       # rows per core (one expert per core pair; 2*CAP=17408 >> E[16384])
NT = 17          # 512-row tiles per core
R = 512          # rows per tile
C = 4            # 128-row chunks per tile
NUM_FREQS = 10
HID = 256
DOUT = 64
NL = 4           # layers -> 3 residual blocks
TWO_PI_F32 = float(np.float32(2 * np.pi))
HALF_PI_F32 = float(np.float32(0.5 * np.pi))
MAGIC_C = float(np.float32(1.5 * 2 ** 23))

_compiled = {}
RUN_KWARGS = {}    # test.py may set e.g. {"trace": True}
LAST_RESULT = []   # test.py reads the BassKernelResults appended here


def _build_program():
    import concourse.bass as bass
    from concourse import bacc
    import concourse.mybir as mybir
    import concourse.tile as tile

    F32 = mybir.dt.float32
    BF16 = mybir.dt.bfloat16
    P = 128
    ALU = mybir.AluOpType
    ACTF = mybir.ActivationFunctionType

    nc = bacc.Bacc("TRN2", target_bir_lowering=False, debug=False)

    # ---- DRAM I/O (per core; one expert's weights) ----
    x_d = nc.dram_tensor("x_rows", [CAP, 4], F32, kind="ExternalInput").ap()
    d_d = nc.dram_tensor("indim_rows", [CAP], F32, kind="ExternalInput").ap()
    fr_d = nc.dram_tensor("fr10", [NUM_FREQS], F32, kind="ExternalInput").ap()
    w0_d = nc.dram_tensor("w0", [85, HID], BF16, kind="ExternalInput").ap()
    wh_d = nc.dram_tensor("wh", [P, NL - 1, 2, 2, P], BF16,
                          kind="ExternalInput").ap()
    wo_d = nc.dram_tensor("wo", [P, 2, DOUT], BF16, kind="ExternalInput").ap()
    b0_d = nc.dram_tensor("b0r", [P, 2], F32, kind="ExternalInput").ap()
    bh_d = nc.dram_tensor("bhr", [P, NL - 1, 2], F32, kind="ExternalInput").ap()
    bo_d = nc.dram_tensor("bor", [DOUT, 1], F32, kind="ExternalInput").ap()
    sc_d = nc.dram_tensor("scal3", [NL - 1], F32, kind="ExternalInput").ap()
    out_d = nc.dram_tensor("out_rows", [CAP, DOUT], F32,
                           kind="ExternalOutput").ap()

    with tile.TileContext(nc) as tc:
        with tc.tile_pool(name="const", bufs=1) as cpool, \
             tc.tile_pool(name="fr", bufs=4) as fpool, \
             tc.tile_pool(name="act", bufs=8) as apool, \
             tc.tile_pool(name="psz", bufs=3, space="PSUM") as zpool, \
             tc.tile_pool(name="pso", bufs=2, space="PSUM") as opool:

            # ---- constants / weights into SBUF (once) ----
            w0 = cpool.tile([85, HID], BF16)
            nc.gpsimd.dma_start(out=w0, in_=w0_d)
            wh = cpool.tile([P, NL - 1, 2, 2, P], BF16)
            nc.gpsimd.dma_start(out=wh, in_=wh_d)
            wo = cpool.tile([P, 2, DOUT], BF16)
            nc.gpsimd.dma_start(out=wo, in_=wo_d)
            b0 = cpool.tile([P, 2], F32)
            nc.gpsimd.dma_start(out=b0, in_=b0_d)
            bh = cpool.tile([P, NL - 1, 2], F32)
            nc.gpsimd.dma_start(out=bh, in_=bh_d)
            bo = cpool.tile([DOUT, 1], F32)
            nc.gpsimd.dma_start(out=bo, in_=bo_d)
            scl = cpool.tile([P, NL - 1], F32)
            nc.gpsimd.dma_start(
                out=scl,
                in_=bass.AP(tensor=sc_d.tensor, offset=0,
                            ap=[[0, P], [1, NL - 1]]))
            fr = cpool.tile([P, NUM_FREQS], F32)
            nc.gpsimd.dma_start(
                out=fr,
                in_=bass.AP(tensor=fr_d.tensor, offset=0,
                            ap=[[0, P], [1, NUM_FREQS]]))
            ph = cpool.tile([P, 2], F32)
            nc.vector.memset(ph[:, 0:1], 0.0)
            nc.vector.memset(ph[:, 1:2], 0.25)
            # s2-prescaled out weights: out = Wo^T h2 + (s2 Wo)^T t3, which
            # removes the third residual STT from the per-tile loop
            wos = cpool.tile([P, 2, DOUT], BF16)
            nc.vector.tensor_scalar_mul(wos, wo, scl[:, 2:3])

            def front(t):
                """Row-major front-end: load, normalize, encode -> xe."""
                r0 = t * R
                x_t = fpool.tile([P, C, 4], F32, tag="x_t")
                nc.sync.dma_start(
                    out=x_t,
                    in_=bass.AP(tensor=x_d.tensor, offset=r0 * 4,
                                ap=[[4, P], [4 * P, C], [1, 4]]))
                d_t = fpool.tile([P, C], F32, tag="d_t")
                nc.sync.dma_start(
                    out=d_t,
                    in_=bass.AP(tensor=d_d.tensor, offset=r0,
                                ap=[[1, P], [P, C]]))

                rc = fpool.tile([P, C], F32, tag="rc")
                nc.vector.reciprocal(rc, x_t[:, :, 3])
                xn = fpool.tile([P, C, 4], F32, tag="xn")
                nc.gpsimd.tensor_mul(xn, x_t,
                                     rc[:, :, None].to_broadcast((P, C, 4)))
                nc.gpsimd.tensor_copy(xn[:, :, 3], x_t[:, :, 3])

                # angles in turns: t20[p, c, j, i] = x'_j * 2^(i-1)  (exact);
                # t40 doubles it with the cos quarter-turn phase (folded in
                # BEFORE range reduction -- the Sin table domain is ~[-pi,pi])
                t20 = fpool.tile([P, C, 4, NUM_FREQS], F32, tag="t20")
                nc.gpsimd.tensor_mul(
                    t20, xn[:, :, :, None].to_broadcast((P, C, 4, NUM_FREQS)),
                    fr[:, None, None, :].to_broadcast((P, C, 4, NUM_FREQS)))
                t20f = t20.rearrange("p c j i -> p c (j i)")
                t40 = fpool.tile([P, C, 2, 40], F32, tag="t40")
                nc.gpsimd.tensor_tensor(
                    t40,
                    t20f[:, :, None, :].to_broadcast((P, C, 2, 40)),
                    ph[:, None, :, None].to_broadcast((P, C, 2, 40)),
                    ALU.add)
                # k = round(t40) via fp32 magic add; m40 = t40 - k in [-.5,.5]
                kt = fpool.tile([P, C, 2, 40], F32, tag="kt")
                nc.vector.tensor_scalar(kt, t40, MAGIC_C, MAGIC_C,
                                        ALU.add, ALU.subtract)
                m40 = fpool.tile([P, C, 2, 40], F32, tag="m40")
                nc.gpsimd.tensor_tensor(m40, t40, kt, ALU.subtract)

                # xe rows: [0:4]=x', [4:44]=sin, [44:84]=cos, [84]=1 (bias
                # row for the l0 matmul), [85:128]=junk
                xe_r = fpool.tile([P, C, P], BF16, tag="xe_r")
                nc.gpsimd.tensor_copy(xe_r[:, :, 0:4], xn)
                nc.gpsimd.memset(xe_r[:, :, 84:85], 1.0)
                m40f = m40.rearrange("p c s f -> p c (s f)")
                nc.scalar.activation(xe_r[:, :, 4:84], m40f, ACTF.Sin,
                                     bias=0.0, scale=TWO_PI_F32)

                # flip to feature-major via DMA XBAR transpose
                xe = apool.tile([P, R], BF16, tag="xe")
                nc.sync.dma_start(out=xe.rearrange("p (c q) -> p c q", c=C),
                                  in_=xe_r, transpose=True)
                return xe, d_t

            def l0_mm(xe):
                # bias rides the ones row, so one biasless relu covers both
                # halves of z0
                z0 = zpool.tile([P, 2, R], F32, tag="z")
                nc.tensor.matmul(z0[:, 0, :], w0[:, 0:P], xe[0:85, :],
                                 start=True, stop=True)
                nc.tensor.matmul(z0[:, 1, :], w0[:, P:HID], xe[0:85, :],
                                 start=True, stop=True)
                return z0

            def l0_relu(z0):
                h = apool.tile([P, 2, R], BF16, tag="h")
                nc.scalar.activation(h, z0, ACTF.Relu, bias=0.0, scale=1.0)
                return h

            def layer_mm(k, h):
                zk = zpool.tile([P, 2, R], F32, tag="z")
                for mb in range(2):
                    nc.tensor.matmul(zk[:, mb, :], wh[:, k, 0, mb, :],
                                     h[:, 0, :], start=True, stop=False)
                    nc.tensor.matmul(zk[:, mb, :], wh[:, k, 1, mb, :],
                                     h[:, 1, :], start=False, stop=True)
                return zk

            def layer_post(k, zk, h):
                # t = relu(zk + bh);  k<2: h' = s_k t + h;  k==2: keep t3
                # (its residual is folded into the prescaled out weights)
                tt = apool.tile([P, 2, R], BF16, tag="t")
                nc.scalar.activation(tt[:, 0, :], zk[:, 0, :], ACTF.Relu,
                                     bias=bh[:, k, 0:1], scale=1.0)
                if k == 1:
                    nc.scalar.activation(tt[:, 1, :], zk[:, 1, :], ACTF.Relu,
                                         bias=bh[:, k, 1:2], scale=1.0)
                else:
                    nc.vector.tensor_scalar(tt[:, 1, :], zk[:, 1, :],
                                            bh[:, k, 1:2], 0.0,
                                            ALU.add, ALU.max)
                if k == 2:
                    return h, tt
                # residual split into halves: the next layer's first matmul
                # only needs half 0, so it can start one DVE op earlier
                h_new = apool.tile([P, 2, R], BF16, tag="h")
                for mb in range(2):
                    nc.vector.scalar_tensor_tensor(
                        h_new[:, mb, :], tt[:, mb, :], scl[:, k:k + 1],
                        h[:, mb, :], ALU.mult, ALU.add)
                return h_new, None

            def out_mm_h2(h2):
                o_ps = opool.tile([DOUT, R], F32, tag="o")
                nc.tensor.matmul(o_ps, wo[:, 0, :], h2[:, 0, :],
                                 start=True, stop=False)
                nc.tensor.matmul(o_ps, wo[:, 1, :], h2[:, 1, :],
                                 start=False, stop=False)
                return o_ps

            def out_mm_t3(o_ps, t3):
                nc.tensor.matmul(o_ps, wos[:, 0, :], t3[:, 0, :],
                                 start=False, stop=False)
                nc.tensor.matmul(o_ps, wos[:, 1, :], t3[:, 1, :],
                                 start=False, stop=True)
                return o_ps

            def epilogue(t, o_ps, d_t):
                r0 = t * R
                oT = fpool.tile([DOUT, R], BF16, tag="oT")
                nc.scalar.activation(oT, o_ps, ACTF.Identity,
                                     bias=bo, scale=1.0)
                # flip back to row-major, divide by in_dim, store
                o_r = fpool.tile([P, C, DOUT], BF16, tag="o_r")
                nc.scalar.dma_start(out=o_r, in_=oT, transpose=True)
                rid = fpool.tile([P, C], F32, tag="rid")
                nc.vector.reciprocal(rid, d_t)
                o_f = fpool.tile([P, C, DOUT], F32, tag="o_f")
                nc.gpsimd.tensor_mul(
                    o_f, o_r, rid[:, :, None].to_broadcast((P, C, DOUT)))
                nc.scalar.dma_start(
                    out=bass.AP(tensor=out_d.tensor, offset=r0 * DOUT,
                                ap=[[DOUT, P], [P * DOUT, C], [1, DOUT]]),
                    in_=o_f)

            # two tiles interleaved per layer so the PE always has a ready
            # matmul burst while the other tile's relu/residual chain runs;
            # the next pair's front-end is emitted before this pair's hidden
            # layers so it fills engine idle during the matmul phases
            pairs = [[t0] if t0 + 1 >= NT else [t0, t0 + 1]
                     for t0 in range(0, NT, 2)]
            st = {}
            for t in pairs[0]:
                st[t] = {}
                st[t]["xe"], st[t]["d"] = front(t)
            for pi, pair in enumerate(pairs):
                for t in pair:
                    st[t]["z"] = l0_mm(st[t]["xe"])
                for t in pair:
                    st[t]["h"] = l0_relu(st[t]["z"])
                if pi + 1 < len(pairs):
                    for t in pairs[pi + 1]:
                        st[t] = {}
                        st[t]["xe"], st[t]["d"] = front(t)
                for k in range(NL - 1):
                    for t in pair:
                        st[t]["zk"] = layer_mm(k, st[t]["h"])
                    if k == 2:
                        # out-layer h2 part fills the PE gap while the k2
                        # relu chain runs on ACT/DVE
                        for t in pair:
                            st[t]["o"] = out_mm_h2(st[t]["h"])
                    for t in pair:
                        st[t]["h"], st[t]["t3"] = layer_post(
                            k, st[t]["zk"], st[t]["h"])
                for t in pair:
                    out_mm_t3(st[t]["o"], st[t]["t3"])
                for t in pair:
                    epilogue(t, st[t]["o"], st[t]["d"])

    nc.compile()
    return nc


def _get_program():
    if "nc" not in _compiled:
        _compiled["nc"] = _build_program()
    return _compiled["nc"]


def _xe_perm():
    """perm[slot] = reference xe column for device slot order
    (slots: 0..3 = x', 4 + j*10 + i = sin, 44 + j*10 + i = cos)."""
    perm = np.zeros(84, np.int64)
    perm[0:4] = np.arange(4)
    for s in range(2):
        for j in range(4):
            for i in range(NUM_FREQS):
                perm[4 + s * 40 + j * 10 + i] = 4 + i * 8 + j * 2 + s
    return perm


def _prep_weights(e, W0, b0, Wh, bh, scal, Wout, bout):
    """Host-side layout transforms (permutation / reshape / cast only)."""
    bf = ml_dtypes.bfloat16
    w0 = np.ascontiguousarray(
        np.vstack([W0[e][_xe_perm()], b0[e][None, :]])).astype(bf)  # [85,256]
    wh = np.ascontiguousarray(
        Wh[e].reshape(NL - 1, 2, 128, 2, 128)
        .transpose(2, 0, 1, 3, 4)).astype(bf)                      # [128,3,2,2,128]
    wo = np.ascontiguousarray(
        Wout[e].reshape(2, 128, DOUT).transpose(1, 0, 2)).astype(bf)
    b0r = np.ascontiguousarray(b0[e].reshape(2, 128).T)            # [128,2]
    bhr = np.ascontiguousarray(
        bh[e].reshape(NL - 1, 2, 128).transpose(2, 0, 1))          # [128,3,2]
    bor = np.ascontiguousarray(bout[e].reshape(DOUT, 1))
    sc3 = np.ascontiguousarray(scal[e])
    fr10 = (2.0 ** (np.arange(NUM_FREQS, dtype=np.float32) - 1.0)).astype(
        np.float32)
    return dict(w0=w0, wh=wh, wo=wo, b0r=b0r, bhr=bhr, bor=bor,
                scal3=sc3, fr10=fr10)


def kernel(x, in_dim, layer_id, W0, b0, Wh, bh, scal, Wout, bout):
    from concourse.bass_utils import run_bass_kernel_spmd

    x = np.asarray(x, np.float32)
    in_dim = np.asarray(in_dim, np.float32)
    layer_id = np.asarray(layer_id)
    W0 = np.asarray(W0, np.float32)
    b0 = np.asarray(b0, np.float32)
    Wh = np.asarray(Wh, np.float32)
    bh = np.asarray(bh, np.float32)
    scal = np.asarray(scal, np.float32)
    Wout = np.asarray(Wout, np.float32)
    bout = np.asarray(bout, np.float32)

    # ---- dispatch: expert e -> cores 2e, 2e+1; pad to CAP per core ----
    PADIDX = N
    x_aug = np.vstack([x, np.ones((1, 4), np.float32)])
    d_aug = np.concatenate([in_dim, np.ones(1, np.float32)])
    perms = np.full((NCORE, CAP), PADIDX, np.int64)
    overflow = []
    for e in range(E):
        idx = np.flatnonzero(layer_id == e)
        if len(idx) > 2 * CAP:
            overflow.append(idx[2 * CAP:])
            idx = idx[:2 * CAP]
        nh = min((len(idx) + 1) // 2, CAP)
        perms[2 * e, :nh] = idx[:nh]
        perms[2 * e + 1, :len(idx) - nh] = idx[nh:]

    in_maps = []
    for c in range(NCORE):
        m = _prep_weights(c // 2, W0, b0, Wh, bh, scal, Wout, bout)
        p = perms[c]
        m["x_rows"] = np.ascontiguousarray(x_aug[p])
        m["indim_rows"] = np.ascontiguousarray(d_aug[p])
        in_maps.append(m)

    nc = _get_program()
    res = run_bass_kernel_spmd(nc, in_maps, core_ids=list(range(NCORE)),
                               **RUN_KWARGS)
    LAST_RESULT.clear()
    LAST_RESULT.append(res)

    out = np.zeros((N + 1, DOUT), np.float32)
    for c in range(NCORE):
        out[perms[c]] = np.asarray(res.results[c]["out_rows"], np.float32)

    # pathological overflow fallback (never hit for the benchmark input)
    if overflow:
        ov = np.concatenate(overflow)
        out[ov] = _numpy_ref(x[ov], in_dim[ov], layer_id[ov], W0, b0, Wh, bh,
                             scal, Wout, bout)
    return out[:N]


def _numpy_ref(x, in_dim, layer_id, W0, b0, Wh, bh, scal, Wout, bout):
    x = np.concatenate([x[:, :3] / x[:, 3:4], x[:, 3:]], axis=1)
    freqs = (2.0 ** np.arange(NUM_FREQS, dtype=np.float32)) * np.float32(np.pi)
    ang = x[:, None, :] * freqs[None, :, None]
    sc = np.stack([np.sin(ang), np.cos(ang)], axis=-1)
    xe = np.concatenate([x, sc.reshape(x.shape[0], -1)], axis=1)
    out = np.zeros((x.shape[0], DOUT), np.float32)
    for e in range(E):
        m = layer_id == e
        if not m.any():
            continue
        h = np.maximum(xe[m] @ W0[e] + b0[e], 0.0)
        for k in range(NL - 1):
            h = scal[e, k] * np.maximum(h @ Wh[e, k] + bh[e, k], 0.0) + h
        out[m] = h @ Wout[e] + bout[e]
    return out / in_dim[:, None]


# revision 27
# speedup vs baseline: 2.6022x; 1.0561x over previous
"""Trainium2 Bass kernel for nn_NeRF_MLP_Compose (MoE-routed NeRF MLP).

Strategy (v2):
  - Host-side MoE dispatch: each expert's rows are split across a PAIR of
    cores (core c handles expert c//2), so each core runs ONE expert dense
    over ~8.2k rows (CAP=8704 padded) and holds only that expert's weights.
  - bf16 weights + activations for all matmuls (tolerance is 2e-2); the
    positional-encoding angle path stays fp32 for phase accuracy.
  - Row-major front-end: normalize + angle/[mod 1]/sin/cos are computed with
    rows on partitions (no PE transposes, no PSUM copies); the encoded
    features are flipped to feature-major with the DMA XBAR transpose
    (16-bit, 16x128 tiles).  Output is flipped back the same way.
  - MLP: feature-major, K<=128 stationary blocks, N=512 moving tiles.
    Third residual folded into the out layer input (h3 = s2*t3 + h2).
  - Element-wise work is spread across ACT / DVE / GPSIMD so each engine's
    per-tile time roughly matches the PE's; the Tile list-scheduler
    overlaps tiles (all pools are multi-buffered).
"""
import sys
for _p in ("/opt/trn_rl_repo", "/root/.axon_site/_ro/trn_rl_repo"):
    if _p not in sys.path:
        sys.path.insert(0, _p)

import numpy as np
import ml_dtypes

N = 65536
E = 4            # experts
NCORE = 8
CAP = 8704       # rows per core (one expert per core pair; 2*CAP=17408 >> E[16384])
NT = 17          # 512-row tiles per core
R = 512          # rows per tile
C = 4            # 128-row chunks per tile
NUM_FREQS = 10
HID = 256
DOUT = 64
NL = 4           # layers -> 3 residual blocks
TWO_PI_F32 = float(np.float32(2 * np.pi))
HALF_PI_F32 = float(np.float32(0.5 * np.pi))
MAGIC_C = float(np.float32(1.5 * 2 ** 23))

_compiled = {}
RUN_KWARGS = {}    # test.py may set e.g. {"trace": True}
LAST_RESULT = []   # test.py reads the BassKernelResults appended here


def _build_program():
    import concourse.bass as bass
    from concourse import bacc
    import concourse.mybir as mybir
    import concourse.tile as tile

    F32 = mybir.dt.float32
    BF16 = mybir.dt.bfloat16
    P = 128
    ALU = mybir.AluOpType
    ACTF = mybir.ActivationFunctionType

    nc = bacc.Bacc("TRN2", target_bir_lowering=False, debug=False)

    # ---- DRAM I/O (per core; one expert's weights) ----
    x_d = nc.dram_tensor("x_rows", [CAP, 4], F32, kind="ExternalInput").ap()
    d_d = nc.dram_tensor("indim_rows", [CAP], F32, kind="ExternalInput").ap()
    fr_d = nc.dram_tensor("fr10", [NUM_FREQS], F32, kind="ExternalInput").ap()
    w0_d = nc.dram_tensor("w0", [85, HID], BF16, kind="ExternalInput").ap()
    wh_d = nc.dram_tensor("wh", [P, NL - 1, 2, 2, P], BF16,
                          kind="ExternalInput").ap()
    wo_d = nc.dram_tensor("wo", [P, 2, DOUT], BF16, kind="ExternalInput").ap()
    b0_d = nc.dram_tensor("b0r", [P, 2], F32, kind="ExternalInput").ap()
    bh_d = nc.dram_tensor("bhr", [P, NL - 1, 2], F32, kind="ExternalInput").ap()
    bo_d = nc.dram_tensor("bor", [DOUT, 1], F32, kind="ExternalInput").ap()
    sc_d = nc.dram_tensor("scal3", [NL - 1], F32, kind="ExternalInput").ap()
    out_d = nc.dram_tensor("out_rows", [CAP, DOUT], F32,
                           kind="ExternalOutput").ap()

    with tile.TileContext(nc) as tc:
        with tc.tile_pool(name="const", bufs=1) as cpool, \
             tc.tile_pool(name="fr", bufs=4) as fpool, \
             tc.tile_pool(name="act", bufs=8) as apool, \
             tc.tile_pool(name="psz", bufs=3, space="PSUM") as zpool, \
             tc.tile_pool(name="pso", bufs=2, space="PSUM") as opool:

            # ---- constants / weights into SBUF (once) ----
            w0 = cpool.tile([85, HID], BF16)
            nc.scalar.dma_start(out=w0, in_=w0_d)
            wh = cpool.tile([P, NL - 1, 2, 2, P], BF16)
            nc.scalar.dma_start(out=wh, in_=wh_d)
            wo = cpool.tile([P, 2, DOUT], BF16)
            nc.scalar.dma_start(out=wo, in_=wo_d)
            b0 = cpool.tile([P, 2], F32)
            nc.scalar.dma_start(out=b0, in_=b0_d)
            bh = cpool.tile([P, NL - 1, 2], F32)
            nc.scalar.dma_start(out=bh, in_=bh_d)
            bo = cpool.tile([DOUT, 1], F32)
            nc.scalar.dma_start(out=bo, in_=bo_d)
            scl = cpool.tile([P, NL - 1], F32)
            nc.scalar.dma_start(
                out=scl,
                in_=bass.AP(tensor=sc_d.tensor, offset=0,
                            ap=[[0, P], [1, NL - 1]]))
            fr = cpool.tile([P, NUM_FREQS], F32)
            nc.scalar.dma_start(
                out=fr,
                in_=bass.AP(tensor=fr_d.tensor, offset=0,
                            ap=[[0, P], [1, NUM_FREQS]]))
            ph = cpool.tile([P, 2], F32)
            nc.vector.memset(ph[:, 0:1], 0.0)
            nc.vector.memset(ph[:, 1:2], 0.25)
            # s2-prescaled out weights: out = Wo^T h2 + (s2 Wo)^T t3, which
            # removes the third residual STT from the per-tile loop
            wos = cpool.tile([P, 2, DOUT], BF16)
            nc.vector.tensor_scalar_mul(wos, wo, scl[:, 2:3])

            def group_load(group):
                """One batched x / in_dim DMA for a whole tile group."""
                r0 = group[0] * R
                cg = C * len(group)
                x_g = fpool.tile([P, len(group), C, 4], F32, tag="x_t",
                                 bufs=2)
                nc.sync.dma_start(
                    out=x_g.rearrange("p g c q -> p (g c) q"),
                    in_=bass.AP(tensor=x_d.tensor, offset=r0 * 4,
                                ap=[[4, P], [4 * P, cg], [1, 4]]))
                d_g = fpool.tile([P, len(group), C], F32, tag="d_t", bufs=2)
                nc.sync.dma_start(
                    out=d_g.rearrange("p g c -> p (g c)"),
                    in_=bass.AP(tensor=d_d.tensor, offset=r0,
                                ap=[[1, P], [P, cg]]))
                return x_g, d_g

            def front(t, x_t, d_t):
                """Row-major front-end: normalize + encode -> xe."""
                rc = fpool.tile([P, C], F32, tag="rc")
                nc.vector.reciprocal(rc, x_t[:, :, 3])
                xn = fpool.tile([P, C, 4], F32, tag="xn")
                nc.gpsimd.tensor_mul(xn, x_t,
                                     rc[:, :, None].to_broadcast((P, C, 4)))
                nc.gpsimd.tensor_copy(xn[:, :, 3], x_t[:, :, 3])

                # angles in turns: t20[p, c, j, i] = x'_j * 2^(i-1)  (exact);
                # t40 doubles it with the cos quarter-turn phase (folded in
                # BEFORE range reduction -- the Sin table domain is ~[-pi,pi])
                t20 = fpool.tile([P, C, 4, NUM_FREQS], F32, tag="t20")
                nc.gpsimd.tensor_mul(
                    t20, xn[:, :, :, None].to_broadcast((P, C, 4, NUM_FREQS)),
                    fr[:, None, None, :].to_broadcast((P, C, 4, NUM_FREQS)))
                t20f = t20.rearrange("p c j i -> p c (j i)")
                t40 = fpool.tile([P, C, 2, 40], F32, tag="t40")
                nc.gpsimd.tensor_tensor(
                    t40,
                    t20f[:, :, None, :].to_broadcast((P, C, 2, 40)),
                    ph[:, None, :, None].to_broadcast((P, C, 2, 40)),
                    ALU.add)
                # k = round(t40) via fp32 magic add; m40 = t40 - k in [-.5,.5]
                kt = fpool.tile([P, C, 2, 40], F32, tag="kt")
                nc.vector.tensor_scalar(kt, t40, MAGIC_C, MAGIC_C,
                                        ALU.add, ALU.subtract)
                m40 = fpool.tile([P, C, 2, 40], F32, tag="m40")
                nc.gpsimd.tensor_tensor(m40, t40, kt, ALU.subtract)

                # xe rows: [0:4]=x', [4:44]=sin, [44:84]=cos, [84]=1 (bias
                # row for the l0 matmul), [85:128]=junk
                xe_r = fpool.tile([P, C, P], BF16, tag="xe_r")
                nc.gpsimd.tensor_copy(xe_r[:, :, 0:4], xn)
                nc.gpsimd.memset(xe_r[:, :, 84:85], 1.0)
                m40f = m40.rearrange("p c s f -> p c (s f)")
                nc.scalar.activation(xe_r[:, :, 4:84], m40f, ACTF.Sin,
                                     bias=0.0, scale=TWO_PI_F32)

                # flip to feature-major via DMA XBAR transpose
                xe = apool.tile([P, R], BF16, tag="xe")
                nc.sync.dma_start(out=xe.rearrange("p (c q) -> p c q", c=C),
                                  in_=xe_r, transpose=True)
                return xe, d_t

            def l0_mm(xe):
                # bias rides the ones row, so one biasless relu covers both
                # halves of z0
                z0 = zpool.tile([P, 2, R], F32, tag="z")
                nc.tensor.matmul(z0[:, 0, :], w0[:, 0:P], xe[0:85, :],
                                 start=True, stop=True)
                nc.tensor.matmul(z0[:, 1, :], w0[:, P:HID], xe[0:85, :],
                                 start=True, stop=True)
                return z0

            def l0_relu(z0):
                h = apool.tile([P, 2, R], BF16, tag="h")
                nc.scalar.activation(h, z0, ACTF.Relu, bias=0.0, scale=1.0)
                return h

            def layer_mm(k, h):
                zk = zpool.tile([P, 2, R], F32, tag="z")
                for mb in range(2):
                    nc.tensor.matmul(zk[:, mb, :], wh[:, k, 0, mb, :],
                                     h[:, 0, :], start=True, stop=False)
                    nc.tensor.matmul(zk[:, mb, :], wh[:, k, 1, mb, :],
                                     h[:, 1, :], start=False, stop=True)
                return zk

            def layer_post(k, zk, h):
                # t = relu(zk + bh);  k<2: h' = s_k t + h;  k==2: keep t3
                # (its residual is folded into the prescaled out weights)
                tt = apool.tile([P, 2, R], BF16, tag="t")
                nc.scalar.activation(tt[:, 0, :], zk[:, 0, :], ACTF.Relu,
                                     bias=bh[:, k, 0:1], scale=1.0)
                if k == 1:
                    nc.scalar.activation(tt[:, 1, :], zk[:, 1, :], ACTF.Relu,
                                         bias=bh[:, k, 1:2], scale=1.0)
                else:
                    nc.vector.tensor_scalar(tt[:, 1, :], zk[:, 1, :],
                                            bh[:, k, 1:2], 0.0,
                                            ALU.add, ALU.max)
                if k == 2:
                    return h, tt
                # residual split into halves: the next layer's first matmul
                # only needs half 0, so it can start one DVE op earlier
                h_new = apool.tile([P, 2, R], BF16, tag="h")
                for mb in range(2):
                    nc.vector.scalar_tensor_tensor(
                        h_new[:, mb, :], tt[:, mb, :], scl[:, k:k + 1],
                        h[:, mb, :], ALU.mult, ALU.add)
                return h_new, None

            def out_mm_h2(h2):
                o_ps = opool.tile([DOUT, R], F32, tag="o")
                nc.tensor.matmul(o_ps, wo[:, 0, :], h2[:, 0, :],
                                 start=True, stop=False)
                nc.tensor.matmul(o_ps, wo[:, 1, :], h2[:, 1, :],
                                 start=False, stop=False)
                return o_ps

            def out_mm_t3(o_ps, t3):
                nc.tensor.matmul(o_ps, wos[:, 0, :], t3[:, 0, :],
                                 start=False, stop=False)
                nc.tensor.matmul(o_ps, wos[:, 1, :], t3[:, 1, :],
                                 start=False, stop=True)
                return o_ps

            def epilogue(t, o_ps, d_t):
                r0 = t * R
                oT = fpool.tile([DOUT, R], BF16, tag="oT")
                nc.scalar.activation(oT, o_ps, ACTF.Identity,
                                     bias=bo, scale=1.0)
                # flip back to row-major, divide by in_dim, store
                o_r = fpool.tile([P, C, DOUT], BF16, tag="o_r")
                nc.scalar.dma_start(out=o_r, in_=oT, transpose=True)
                rid = fpool.tile([P, C], F32, tag="rid")
                nc.vector.reciprocal(rid, d_t)
                o_f = fpool.tile([P, C, DOUT], F32, tag="o_f")
                nc.gpsimd.tensor_mul(
                    o_f, o_r, rid[:, :, None].to_broadcast((P, C, DOUT)))
                nc.scalar.dma_start(
                    out=bass.AP(tensor=out_d.tensor, offset=r0 * DOUT,
                                ap=[[DOUT, P], [P * DOUT, C], [1, DOUT]]),
                    in_=o_f)

            # two tiles interleaved per layer so the PE always has a ready
            # matmul burst while the other tile's relu/residual chain runs;
            # the next group's front-end is emitted before this group's
            # hidden layers so it fills engine idle during the matmul phases
            groups = [[t0, t0 + 1] for t0 in range(0, NT - 3, 2)]
            groups.append(list(range(NT - 3, NT)))  # last: 3-way interleave

            def fronts(group):
                x_g, d_g = group_load(group)
                for gi, t in enumerate(group):
                    st[t] = {}
                    st[t]["xe"], st[t]["d"] = front(t, x_g[:, gi], d_g[:, gi])

            st = {}
            fronts(groups[0])
            for pi, group in enumerate(groups):
                for t in group:
                    st[t]["z"] = l0_mm(st[t]["xe"])
                for t in group:
                    st[t]["h"] = l0_relu(st[t]["z"])
                if pi + 1 < len(groups):
                    fronts(groups[pi + 1])
                for k in range(NL - 1):
                    for t in group:
                        st[t]["zk"] = layer_mm(k, st[t]["h"])
                    if k == 2:
                        # out-layer h2 part fills the PE gap while the k2
                        # relu chain runs on ACT/DVE
                        for t in group:
                            st[t]["o"] = out_mm_h2(st[t]["h"])
                    for t in group:
                        st[t]["h"], st[t]["t3"] = layer_post(
                            k, st[t]["zk"], st[t]["h"])
                for t in group:
                    out_mm_t3(st[t]["o"], st[t]["t3"])
                for t in group:
                    epilogue(t, st[t]["o"], st[t]["d"])

    nc.compile()
    return nc


def _get_program():
    if "nc" not in _compiled:
        _compiled["nc"] = _build_program()
    return _compiled["nc"]


def _xe_perm():
    """perm[slot] = reference xe column for device slot order
    (slots: 0..3 = x', 4 + j*10 + i = sin, 44 + j*10 + i = cos)."""
    perm = np.zeros(84, np.int64)
    perm[0:4] = np.arange(4)
    for s in range(2):
        for j in range(4):
            for i in range(NUM_FREQS):
                perm[4 + s * 40 + j * 10 + i] = 4 + i * 8 + j * 2 + s
    return perm


def _prep_weights(e, W0, b0, Wh, bh, scal, Wout, bout):
    """Host-side layout transforms (permutation / reshape / cast only)."""
    bf = ml_dtypes.bfloat16
    w0 = np.ascontiguousarray(
        np.vstack([W0[e][_xe_perm()], b0[e][None, :]])).astype(bf)  # [85,256]
    wh = np.ascontiguousarray(
        Wh[e].reshape(NL - 1, 2, 128, 2, 128)
        .transpose(2, 0, 1, 3, 4)).astype(bf)                      # [128,3,2,2,128]
    wo = np.ascontiguousarray(
        Wout[e].reshape(2, 128, DOUT).transpose(1, 0, 2)).astype(bf)
    b0r = np.ascontiguousarray(b0[e].reshape(2, 128).T)            # [128,2]
    bhr = np.ascontiguousarray(
        bh[e].reshape(NL - 1, 2, 128).transpose(2, 0, 1))          # [128,3,2]
    bor = np.ascontiguousarray(bout[e].reshape(DOUT, 1))
    sc3 = np.ascontiguousarray(scal[e])
    fr10 = (2.0 ** (np.arange(NUM_FREQS, dtype=np.float32) - 1.0)).astype(
        np.float32)
    return dict(w0=w0, wh=wh, wo=wo, b0r=b0r, bhr=bhr, bor=bor,
                scal3=sc3, fr10=fr10)


def kernel(x, in_dim, layer_id, W0, b0, Wh, bh, scal, Wout, bout):
    from concourse.bass_utils import run_bass_kernel_spmd

    x = np.asarray(x, np.float32)
    in_dim = np.asarray(in_dim, np.float32)
    layer_id = np.asarray(layer_id)
    W0 = np.asarray(W0, np.float32)
    b0 = np.asarray(b0, np.float32)
    Wh = np.asarray(Wh, np.float32)
    bh = np.asarray(bh, np.float32)
    scal = np.asarray(scal, np.float32)
    Wout = np.asarray(Wout, np.float32)
    bout = np.asarray(bout, np.float32)

    # ---- dispatch: expert e -> cores 2e, 2e+1; pad to CAP per core ----
    PADIDX = N
    x_aug = np.vstack([x, np.ones((1, 4), np.float32)])
    d_aug = np.concatenate([in_dim, np.ones(1, np.float32)])
    perms = np.full((NCORE, CAP), PADIDX, np.int64)
    overflow = []
    for e in range(E):
        idx = np.flatnonzero(layer_id == e)
        if len(idx) > 2 * CAP:
            overflow.append(idx[2 * CAP:])
            idx = idx[:2 * CAP]
        nh = min((len(idx) + 1) // 2, CAP)
        perms[2 * e, :nh] = idx[:nh]
        perms[2 * e + 1, :len(idx) - nh] = idx[nh:]

    in_maps = []
    for c in range(NCORE):
        m = _prep_weights(c // 2, W0, b0, Wh, bh, scal, Wout, bout)
        p = perms[c]
        m["x_rows"] = np.ascontiguousarray(x_aug[p])
        m["indim_rows"] = np.ascontiguousarray(d_aug[p])
        in_maps.append(m)

    nc = _get_program()
    res = run_bass_kernel_spmd(nc, in_maps, core_ids=list(range(NCORE)),
                               **RUN_KWARGS)
    LAST_RESULT.clear()
    LAST_RESULT.append(res)

    out = np.zeros((N + 1, DOUT), np.float32)
    for c in range(NCORE):
        out[perms[c]] = np.asarray(res.results[c]["out_rows"], np.float32)

    # pathological overflow fallback (never hit for the benchmark input)
    if overflow:
        ov = np.concatenate(overflow)
        out[ov] = _numpy_ref(x[ov], in_dim[ov], layer_id[ov], W0, b0, Wh, bh,
                             scal, Wout, bout)
    return out[:N]


def _numpy_ref(x, in_dim, layer_id, W0, b0, Wh, bh, scal, Wout, bout):
    x = np.concatenate([x[:, :3] / x[:, 3:4], x[:, 3:]], axis=1)
    freqs = (2.0 ** np.arange(NUM_FREQS, dtype=np.float32)) * np.float32(np.pi)
    ang = x[:, None, :] * freqs[None, :, None]
    sc = np.stack([np.sin(ang), np.cos(ang)], axis=-1)
    xe = np.concatenate([x, sc.reshape(x.shape[0], -1)], axis=1)
    out = np.zeros((x.shape[0], DOUT), np.float32)
    for e in range(E):
        m = layer_id == e
        if not m.any():
            continue
        h = np.maximum(xe[m] @ W0[e] + b0[e], 0.0)
        for k in range(NL - 1):
            h = scal[e, k] * np.maximum(h @ Wh[e, k] + bh[e, k], 0.0) + h
        out[m] = h @ Wout[e] + bout[e]
    return out / in_dim[:, None]
